# revision 30
# baseline (speedup 1.0000x reference)
"""Multi-head attention (RoPE + u-bias + bool mask) Trainium2 Bass kernel.

Contract: kernel(**inputs) takes FULL unsharded inputs (see shapes below),
shards batch across 8 NeuronCores (data parallel), runs one Bass/Tile
program per core, and gathers the full output.

Hardcoded problem shapes:
  query/key/value: (8, 1024, 1024) f32, mask: (8, 1024, 1024) bool,
  Wq/Wk/Wv/Wo: (1024, 1024) f32, bq/bk/bv/bo: (1024,) f32,
  u_bias: (16, 64) f32.  Output: (8, 1024, 1024) f32.

v3: bf16 matmul datapath (fp32 PSUM accumulate); all HBM traffic as flat
[128, 8192] contiguous DMAs (host pre-transposes into SBUF layout); softmax
denominator kept on-chip (partition broadcast via selector matmul); merged
PSUM pools so stages A+B and C+C2+D overlap; evacuations balanced across
ACT (partition-shifting copies + exp) and DVE (everything else).
"""

import sys

if "/opt/trn_rl_repo" not in sys.path:
    sys.path.insert(0, "/opt/trn_rl_repo")

from contextlib import ExitStack

import ml_dtypes
import numpy as np

import concourse.bass as bass
from concourse import bacc
import concourse.tile as tile
from concourse import mybir
from concourse.bass_utils import run_bass_kernel_spmd

B, S, D, H, Dh = 8, 1024, 1024, 16, 64
P = 128
NT = D // P  # 8 partition-tiles along d
ST = S // P  # 8 tiles along s/t
HF = S // 2  # 512 = matmul moving chunk
FP = mybir.dt.float32
BF = mybir.dt.bfloat16
ROPE_BASE = 10000.0
AF = mybir.ActivationFunctionType
ALU = mybir.AluOpType

N_CORES = 8
import os

STAGES = int(os.environ.get("K_STAGES", "5"))
# mask-multiply tiles with (tt % 8) >= GPS_TT go to GPSIMD instead of DVE
GPS_TT = int(os.environ.get("K_GPS_TT", "8"))

# column offsets inside the packed mega input tensor
_SIZES = [
    ("xq", NT * S), ("xk", NT * S), ("xv", NT * S),
    ("wq", NT * D), ("wk", NT * D), ("wv", NT * D), ("wo", NT * D),
    ("mask", ST * S), ("cs", 2 * S), ("pswap", P), ("smalls", 24),
    ("sel16", NT * P), ("rows", 2 * D + P),
]
OFF = {}
_o = 0
for _n, _s in _SIZES:
    OFF[_n] = _o
    _o += _s
MEGA_COLS = _o


def build_nc():
    nc = bacc.Bacc("TRN2", target_bir_lowering=False, debug=False)

    # Single mega input: every tensor packed into one [P, MEGA_COLS] bf16
    # DRAM tensor (per-input dispatch overhead through the PJRT tunnel is
    # ~70us/tensor, so one input instead of 14 dominates the bench time).
    # Host prearranges big tensors to [P, NT*X] so each DMA slice is a
    # flat contiguous copy (16KB per partition row).
    mega = nc.dram_tensor("mega", [P, MEGA_COLS], BF, kind="ExternalInput").ap()
    xq_d = mega[:, OFF["xq"] : OFF["xq"] + NT * S]
    xk_d = mega[:, OFF["xk"] : OFF["xk"] + NT * S]
    xv_d = mega[:, OFF["xv"] : OFF["xv"] + NT * S]
    wq_d = mega[:, OFF["wq"] : OFF["wq"] + NT * D]
    wk_d = mega[:, OFF["wk"] : OFF["wk"] + NT * D]
    wv_d = mega[:, OFF["wv"] : OFF["wv"] + NT * D]
    wo_d = mega[:, OFF["wo"] : OFF["wo"] + NT * D]
    mask_d = mega[:, OFF["mask"] : OFF["mask"] + ST * S]
    # cs[:, 0:S] = cos table, cs[:, S:2S] = sign-folded sin table
    cs_d = mega[:, OFF["cs"] : OFF["cs"] + 2 * S]
    # smalls[:, 0:8]=u cols, 8:16=bq cols, 16:24=bk cols
    smalls_d = mega[:, OFF["smalls"] : OFF["smalls"] + 24]
    pswap_d = mega[:, OFF["pswap"] : OFF["pswap"] + P]
    # rows[0, 0:D]=bv, D:2D=bo, 2D:2D+P=ones
    rows_d = mega[0:1, OFF["rows"] : OFF["rows"] + 2 * D + P]
    # sel16: [16, NT*P]; block j: row 2j = 1 on cols 0:64, row 2j+1 on 64:128
    sel16_d = mega[0:H, OFF["sel16"] : OFF["sel16"] + NT * P]
    out_d = nc.dram_tensor("out", [S, D], FP, kind="ExternalOutput").ap()

    with tile.TileContext(nc) as tc, ExitStack() as ctx:
        persist = ctx.enter_context(tc.tile_pool(name="persist", bufs=1))

        # ---- persistent constants / state ----
        smalls_sb = persist.tile([P, 24], BF)
        ucols = smalls_sb[:, 0:8]
        bqcols = smalls_sb[:, 8:16]
        bkcols = smalls_sb[:, 16:24]
        pswap_sb = persist.tile([P, P], BF)
        rows_sb = persist.tile([1, 2 * D + P], BF)
        bvrow = rows_sb[:, 0:D]
        borow = rows_sb[:, D : 2 * D]
        ones_row = rows_sb[:, 2 * D : 2 * D + P]
        sel16_sb = persist.tile([H, NT * P], BF)

        qb = persist.tile([P, NT * S], BF)  # rope(Q)^T + u, tile j at col j*S
        kb = persist.tile([P, NT * S], BF)  # rope(K)^T
        # V augmented with ones column: [p, tt, h, 0:64]=V, [.,.,.,64]=1
        vaug = persist.tile([P, ST * H * (Dh + 1)], BF)
        vaug_v = vaug[:].rearrange("p (st h c) -> p st h c", st=ST, h=H)
        nc.vector.memset(vaug_v[:, :, :, Dh : Dh + 1], 1.0)
        ctxu = persist.tile([P, NT * S], BF)  # ctx^T (normalized in place)
        den = persist.tile([H, S], FP)  # den[h, s]
        rec = persist.tile([H, S], BF)  # 1/den
        mask_sb = persist.tile([P, ST * S], BF)

        # ======== stages A+B: QKV projections + rope (shared pools) ========
        with (
            tc.tile_pool(name="poolAB", bufs=1) as pA,
            tc.tile_pool(name="psAB", bufs=1, space="PSUM") as psA,
        ):
            # critical-first DMA order: the first projection group needs the
            # full wq + the xq c-half before any matmul can issue; the small
            # constants are consumed ~2us later by the rope tail.
            cs_sb = pA.tile([P, 2 * S], BF, tag="cs")
            cos_sb = cs_sb[:, 0:S]
            sin_sb = cs_sb[:, S : 2 * S]
            wq_sb = pA.tile([P, NT * D], BF, tag="wq")
            nc.sync.dma_start(wq_sb[:], wq_d[:])
            nc.sync.dma_start(pswap_sb[:], pswap_d[:])
            nc.sync.dma_start(smalls_sb[:], smalls_d[:])
            xq_sb = pA.tile([P, NT * S], BF, tag="xq")
            nc.sync.dma_start(xq_sb[:], xq_d[:])
            nc.sync.dma_start(cs_sb[:], cs_d[:])
            wk_sb = pA.tile([P, NT * D], BF, tag="wk")
            nc.sync.dma_start(wk_sb[:], wk_d[:])
            xk_sb = pA.tile([P, NT * S], BF, tag="xk")
            nc.sync.dma_start(xk_sb[:], xk_d[:])
            nc.sync.dma_start(rows_sb[:], rows_d[:])
            nc.sync.dma_start(sel16_sb[:], sel16_d[:])
            wv_sb = pA.tile([P, NT * D], BF, tag="wv")
            nc.sync.dma_start(wv_sb[:], wv_d[:])
            xv_sb = pA.tile([P, NT * S], BF, tag="xv")
            nc.sync.dma_start(xv_sb[:], xv_d[:])
            # mask is only needed in stage C; last in the DMA queue
            nc.sync.dma_start(mask_sb[:], mask_d[:])

            # A-stage iterations, software-pipelined one deep: the rope tail
            # (qp matmul + DVE ops) of iteration i is emitted after the raw
            # matmuls of iteration i+1 so the in-order PE never waits on the
            # ACT bias-evacuation.
            a_iters = [
                (x_sb, w_sb, bcols, is_q, c, j)
                for x_sb, w_sb, bcols, is_q in (
                    (xq_sb, wq_sb, bqcols, True),
                    (xk_sb, wk_sb, bkcols, False),
                )
                for c in range(2)
                for j in range(NT)
            ]
            a_state = {}

            def emit_raw(i):
                x_sb, w_sb, bcols, is_q, c, j = a_iters[i]
                raw = psA.tile([P, HF], FP, tag="raw", bufs=3, name=f"raw{i}")
                for k in range(NT):
                    nc.tensor.matmul(
                        raw[:],
                        w_sb[:, k * D + j * P : k * D + (j + 1) * P],
                        x_sb[:, k * S + c * HF : k * S + (c + 1) * HF],
                        start=(k == 0),
                        stop=(k == NT - 1),
                    )
                # evacuate with per-partition bias (pre-rope); ACT is idle
                # in the A/B window so it takes the evacuations
                q_raw = pA.tile([P, HF], BF, tag="qraw", bufs=3)
                nc.scalar.activation(
                    q_raw[:], raw[:], AF.Identity, bias=bcols[:, j : j + 1]
                )
                a_state[i] = q_raw

            def emit_rope(i):
                x_sb, w_sb, bcols, is_q, c, j = a_iters[i]
                chalf = slice(c * HF, (c + 1) * HF)
                q_raw = a_state.pop(i)
                # partner-swap via permutation matmul
                qp = psA.tile([P, HF], FP, tag="qp", bufs=2)
                nc.tensor.matmul(qp[:], pswap_sb[:], q_raw[:], start=True, stop=True)
                t1 = pA.tile([P, HF], BF, tag="t1", bufs=2)
                nc.vector.tensor_tensor(t1[:], q_raw[:], cos_sb[:, chalf], op=ALU.mult)
                t2 = pA.tile([P, HF], BF, tag="t2", bufs=2)
                nc.vector.tensor_tensor(t2[:], qp[:], sin_sb[:, chalf], op=ALU.mult)
                dst_all = qb if is_q else kb
                dslice = dst_all[:, j * S + c * HF : j * S + (c + 1) * HF]
                if is_q:
                    nc.vector.scalar_tensor_tensor(
                        dslice, t1[:], ucols[:, j : j + 1], t2[:],
                        op0=ALU.add, op1=ALU.add,
                    )
                else:
                    nc.vector.tensor_tensor(dslice, t1[:], t2[:], op=ALU.add)

            for i in range(len(a_iters) + 1):
                if i < len(a_iters):
                    emit_raw(i)
                if i >= 1:
                    emit_rope(i - 1)

            # ---- stage B: V projection into vaug (same pools, overlaps A) ----
            if STAGES >= 3:
                for st in range(ST):
                    for c in range(2):
                        vp = psA.tile([P, HF], FP, tag="vp", bufs=2)
                        for k in range(NT):
                            nc.tensor.matmul(
                                vp[:],
                                xv_sb[:, k * S + st * P : k * S + (st + 1) * P],
                                wv_sb[:, k * D + c * HF : k * D + (c + 1) * HF],
                                start=(k == 0),
                                stop=False,
                            )
                        nc.tensor.matmul(
                            vp[:], ones_row, bvrow[:, c * HF : (c + 1) * HF],
                            start=False, stop=True,
                        )
                        nc.scalar.copy(
                            vaug_v[:, st, c * 8 : (c + 1) * 8, 0:Dh],
                            vp[:].rearrange("p (h c) -> p h c", h=8),
                        )

        # ======== stages C + C2 + D (shared pools) ========
        if STAGES >= 4:
            with (
                tc.tile_pool(name="poolCD", bufs=1) as pC,
                tc.tile_pool(name="psCD", bufs=1, space="PSUM") as psC,
            ):
                wo_sb = pC.tile([P, NT * D], BF, tag="wo")
                nc.sync.dma_start(wo_sb[:], wo_d[:])
                # software pipeline: emit scores for unit u+1 before ctx of
                # unit u, so the in-order PE never blocks on the exp/mask
                # chain of the unit it just scored.
                units = [(j, tt, hi) for j in range(NT) for tt in range(ST) for hi in range(2)]
                cps_tiles = {}
                em_tiles = {}

                def emit_scores(u):
                    j, tt, hi = units[u]
                    half = hi * Dh
                    if hi == 0 and tt == 0:
                        cps_tiles[j] = [
                            psC.tile([Dh + 1, S], FP, tag="ctx", bufs=2, name=f"cps{j}_{k}")
                            for k in range(2)
                        ]
                    sps = psC.tile([P, S], FP, tag="scores", bufs=2, name=f"sps{u}")
                    for c in range(2):
                        nc.tensor.matmul(
                            sps[:, c * HF : (c + 1) * HF],
                            kb[half : half + Dh, j * S + tt * P : j * S + (tt + 1) * P],
                            qb[half : half + Dh, j * S + c * HF : j * S + (c + 1) * HF],
                            start=True,
                            stop=True,
                        )
                    et = pC.tile([P, S], BF, tag="expt", bufs=4)
                    nc.scalar.activation(et[:], sps[:], AF.Exp, scale=0.125)
                    em = pC.tile([P, S], BF, tag="expm", bufs=4)
                    eng = nc.gpsimd if (tt % 8 >= GPS_TT) else nc.vector
                    eng.tensor_tensor(
                        em[:], et[:], mask_sb[:, tt * S : (tt + 1) * S], op=ALU.mult
                    )
                    em_tiles[u] = em

                def emit_ctx(u):
                    j, tt, hi = units[u]
                    h = 2 * j + hi
                    em = em_tiles.pop(u)
                    for c in range(2):
                        nc.tensor.matmul(
                            cps_tiles[j][hi][:, c * HF : (c + 1) * HF],
                            vaug_v[:, tt, h, :],
                            em[:, c * HF : (c + 1) * HF],
                            start=(tt == 0),
                            stop=(tt == ST - 1),
                        )
                    if tt == ST - 1:
                        half = hi * Dh
                        cp = cps_tiles[j][hi]
                        # split the two copies across ACT and DVE so the cps
                        # slot release drains two queues in parallel
                        eng_a = nc.scalar if hi == 0 else nc.vector
                        eng_b = nc.vector if hi == 0 else nc.scalar
                        if eng_a is nc.scalar:
                            eng_a.copy(
                                ctxu[half : half + Dh, j * S : (j + 1) * S],
                                cp[0:Dh, :],
                            )
                        else:
                            eng_a.tensor_copy(
                                ctxu[half : half + Dh, j * S : (j + 1) * S],
                                cp[0:Dh, :],
                            )
                        # engines can only start at partitions {0,32,64,96}:
                        # stage den row at partition 0, DMA-shift to row h
                        dstage = pC.tile([1, S], FP, tag="dstage", bufs=2)
                        if eng_b is nc.scalar:
                            eng_b.copy(dstage[:], cp[Dh : Dh + 1, :])
                        else:
                            eng_b.tensor_copy(dstage[:], cp[Dh : Dh + 1, :])
                        nc.sync.dma_start(den[h : h + 1, :], dstage[:])

                nu = len(units)
                for u in range(nu + 1):
                    if u < nu:
                        emit_scores(u)
                    if u >= 1:
                        emit_ctx(u - 1)
                    # split reciprocal: heads 0..7 are final after j=3
                    if u == 4 * 16:
                        with nc.allow_low_precision(reason="bf16 1/den"):
                            nc.vector.reciprocal(rec[0:8, :], den[0:8, :])

                # ---- stage C2: normalize ctxu in place ----
                with nc.allow_low_precision(reason="bf16 1/den, ~0.4% quant"):
                    nc.vector.reciprocal(rec[:], den[:])
                for j in range(NT):
                    rb = psC.tile([P, S], FP, tag="scores", bufs=2, name=f"rb{j}")
                    for c in range(2):
                        nc.tensor.matmul(
                            rb[:, c * HF : (c + 1) * HF],
                            sel16_sb[:, j * P : (j + 1) * P],
                            rec[:, c * HF : (c + 1) * HF],
                            start=True,
                            stop=True,
                        )
                    nc.vector.tensor_tensor(
                        ctxu[:, j * S : (j + 1) * S],
                        ctxu[:, j * S : (j + 1) * S],
                        rb[:],
                        op=ALU.mult,
                    )

                # ---- stage D: output projection ----
                if STAGES >= 5:
                    for st in range(ST):
                        op = psC.tile([P, S], FP, tag="scores", bufs=2, name=f"op{st}")
                        for c in range(2):
                            for k in range(NT):
                                nc.tensor.matmul(
                                    op[:, c * HF : (c + 1) * HF],
                                    ctxu[:, k * S + st * P : k * S + (st + 1) * P],
                                    wo_sb[:, k * D + c * HF : k * D + (c + 1) * HF],
                                    start=(k == 0),
                                    stop=False,
                                )
                            nc.tensor.matmul(
                                op[:, c * HF : (c + 1) * HF],
                                ones_row,
                                borow[:, c * HF : (c + 1) * HF],
                                start=False,
                                stop=True,
                            )
                        ot = pC.tile([P, S], FP, tag="ot", bufs=2)
                        nc.vector.tensor_copy(ot[:], op[:])
                        nc.sync.dma_start(out_d[st * P : (st + 1) * P, :], ot[:])

    nc.compile()
    return nc


def _to_sb(m):
    """[NT*P, X] -> [P, NT*X] SBUF layout (partition p holds rows p, P+p, ...)."""
    r, x = m.shape
    return np.ascontiguousarray(
        m.reshape(NT, P, x).transpose(1, 0, 2).reshape(P, NT * x)
    )


def _host_consts():
    inv_freq = 1.0 / (ROPE_BASE ** (np.arange(0, Dh, 2, dtype=np.float64) / Dh))
    freqs = np.arange(S, dtype=np.float64)[:, None] * inv_freq[None, :]  # [S, 32]
    cos_rep = np.repeat(np.cos(freqs), 2, axis=-1)  # [S, 64]
    sin_rep = np.repeat(np.sin(freqs), 2, axis=-1)
    costab = np.empty((P, S), np.float32)
    sintab = np.empty((P, S), np.float32)
    for p in range(P):
        dl = p % Dh
        costab[p, :] = cos_rep[:, dl]
        sgn = -1.0 if (p % 2 == 0) else 1.0
        sintab[p, :] = sgn * sin_rep[:, dl]
    pswap = np.zeros((P, P), np.float32)
    for k in range(P):
        pswap[k, k ^ 1] = 1.0
    sel16 = np.zeros((H, NT * P), np.float32)
    for j in range(NT):
        sel16[2 * j, j * P : j * P + Dh] = 1.0
        sel16[2 * j + 1, j * P + Dh : (j + 1) * P] = 1.0
    return costab, sintab, pswap, sel16


_CONSTS = {}


def host_in_maps(query, key, value, mask, Wq, bq, Wk, bk, Wv, bv, u_bias, Wo, bo):
    bf = ml_dtypes.bfloat16
    if not _CONSTS:
        costab, sintab, pswap, sel16 = _host_consts()
        _CONSTS["cs"] = np.concatenate([costab, sintab], axis=1)
        _CONSTS["pswap"] = pswap
        _CONSTS["sel16"] = sel16
    u = np.asarray(u_bias, np.float32)
    smalls = np.zeros((P, 24), np.float32)
    for j in range(NT):
        smalls[:, j] = np.concatenate([u[2 * j], u[2 * j + 1]])
    smalls[:, 8:16] = np.asarray(bq, np.float32).reshape(NT, P).T
    smalls[:, 16:24] = np.asarray(bk, np.float32).reshape(NT, P).T
    rows = np.zeros((P, 2 * D + P), np.float32)
    rows[0] = np.concatenate(
        [np.asarray(bv, np.float32), np.asarray(bo, np.float32), np.ones(P, np.float32)]
    )
    sel16_pad = np.zeros((P, NT * P), np.float32)
    sel16_pad[0:H] = _CONSTS["sel16"]

    def pack(pieces):
        m = np.empty((P, MEGA_COLS), bf)
        for name, arr in pieces.items():
            o = OFF[name]
            m[:, o : o + arr.shape[1]] = arr.astype(bf)
        return m

    shared = dict(
        wq=_to_sb(np.asarray(Wq, np.float32).T.astype(bf)),
        wk=_to_sb(np.asarray(Wk, np.float32).T.astype(bf)),
        wv=_to_sb(np.asarray(Wv, np.float32).T.astype(bf)),
        wo=_to_sb(np.asarray(Wo, np.float32).T.astype(bf)),
        cs=_CONSTS["cs"],
        smalls=smalls,
        pswap=_CONSTS["pswap"],
        rows=rows,
        sel16=sel16_pad,
    )
    in_maps = []
    for b in range(N_CORES):
        pieces = dict(shared)
        pieces["xq"] = _to_sb(np.asarray(query[b], np.float32).T.astype(bf))
        pieces["xk"] = _to_sb(np.asarray(key[b], np.float32).T.astype(bf))
        pieces["xv"] = _to_sb(np.asarray(value[b], np.float32).T.astype(bf))
        pieces["mask"] = _to_sb((~np.asarray(mask[b], bool)).T.astype(bf))
        in_maps.append(dict(mega=pack(pieces)))
    return in_maps


_CACHED = {}


def kernel(query, key, value, mask, Wq, bq, Wk, bk, Wv, bv, u_bias, Wo, bo):
    if "nc" not in _CACHED:
        _CACHED["nc"] = build_nc()
    nc = _CACHED["nc"]
    in_maps = host_in_maps(
        query, key, value, mask, Wq, bq, Wk, bk, Wv, bv, u_bias, Wo, bo
    )
    res = run_bass_kernel_spmd(nc, in_maps, list(range(N_CORES)))
    return np.stack([res.results[b]["out"] for b in range(N_CORES)], axis=0)
